# revision 10
# baseline (speedup 1.0000x reference)
"""Complex-valued multi-head attention (B=4, S=1024, D=128, H=8) on 8 TRN2 cores.

Sharding: tensor-parallel over heads -- one head per NeuronCore. Each core
computes its head's complex Q/K/V projections, complex-magnitude-softmax
attention, and the partial W_o projection for its head slice; the host sums
the 8 partial outputs (the W_o contraction over heads).

Per-core dataflow (fp32 storage, float32r matmuls -- 1 cycle/row on the PE
at moving free-dim >= 256, vs 4 for fp32):
  - Host packs x^T tensors [D, B*S] and per-head weight slices (transposed,
    pre-negated / concatenated) into two DRAM tensors (xall, wpack).
  - Q^T, K^T computed as [d, {r,i}, s] via lhsT=W^T, rhs=x^T; -K_i^T by a
    GPSIMD negate.  V computed natural [s, d] as [V_r | V_i | 1 | 1] rows
    (the ones columns make the attention matmul also produce the softmax
    normalizer Z; two of them because fp32r needs even free dims).
  - Scores computed transposed S^T[k, q] so E = exp(|s|/sqrt(D)) lands in
    the layout attn@V needs as lhsT.  |s|^2 = sr^2 + si^2 split across
    engines: ACT squares sr out of PSUM into the strip, DVE copies si out,
    GPSIMD squares and adds; ACT then does sqrt (input scale 1/D folded in)
    and exp in-place over per-batch strips (one act-table load per function).
  - attn@V: psum[q, 258] accumulates [O_r | O_i | Z | Z] over k chunks.
  - O transposed 128x128 via TensorE, W_o applied, and the 1/Z normalization
    applied via per-partition scale on the output copy.
  - Software pipelining: each (b, qt) unit's attention is deferred two units
    and interleaved between the next batch's score halves, so PE always has
    score/projection matmuls to run while ACT/DVE work through a unit's
    softmax chain.
"""

import numpy as np

import concourse.bacc as bacc
import concourse.mybir as mybir
import concourse.tile as tile
from concourse.bass_utils import run_bass_kernel_spmd

B, S, D, H = 4, 1024, 128, 8
BS = B * S
P = 128
F32 = mybir.dt.float32
F32R = mybir.dt.float32r

X_NAMES = ("xqr", "xqi", "xkr", "xki", "xvr", "xvi")
W1_NAMES = ("wqr", "wqi", "nwqi", "wkr", "wki", "nwki")
W2_NAMES = ("vc1", "vc2", "oc1", "oc2")


W1_OFF = {nm: i * P for i, nm in enumerate(W1_NAMES)}
W2_OFF = {nm: 6 * P + i * 2 * P for i, nm in enumerate(W2_NAMES)}
ID_OFF = 6 * P + 4 * 2 * P          # 1792
ONES_OFF = ID_OFF + P               # 1920
WPACK_COLS = ONES_OFF + 16          # 1936


def build_nc():
    nc = bacc.Bacc()
    xall = nc.dram_tensor("xall", [P, 8 * 6 * 512], F32R, kind="ExternalInput")
    wpack = nc.dram_tensor("wpack", [P, WPACK_COLS], F32R, kind="ExternalInput")
    y = nc.dram_tensor("y", [BS, 2 * P], F32, kind="ExternalOutput")
    xv = xall.rearrange("p (t n c) -> p t n c", t=8, n=6)

    AF = mybir.ActivationFunctionType
    MUL = mybir.AluOpType.mult

    with tile.TileContext(nc) as tc:
        with (
            tc.tile_pool(name="consts", bufs=1) as consts,
            tc.tile_pool(name="xp", bufs=3) as xp,
            tc.tile_pool(name="qk", bufs=2) as qk,
            tc.tile_pool(name="vp", bufs=2) as vp,
            tc.tile_pool(name="sp", bufs=2) as sp,
            tc.tile_pool(name="sc", bufs=3) as sc,
            tc.tile_pool(name="op", bufs=4) as op,
            tc.tile_pool(name="yp", bufs=2) as yp,
            tc.tile_pool(name="rp", bufs=8) as rp,
            tc.tile_pool(name="ps", bufs=2, space="PSUM") as ps,
            tc.tile_pool(name="po", bufs=2, space="PSUM") as po,
            tc.tile_pool(name="pt", bufs=1, space="PSUM") as pt,
            tc.tile_pool(name="py", bufs=1, space="PSUM") as py,
        ):
            wp = consts.tile([P, WPACK_COLS], F32R, name="wp")
            nc.sync.dma_start(wp[:, 0:768], wpack[:, 0:768])
            xt00 = xp.tile([P, 6, 512], F32R, name="xt", tag="xt")
            for dd in range(3):
                nc.sync.dma_start(
                    xt00[:, 2 * dd : 2 * dd + 2, :], xv[:, 0, 2 * dd : 2 * dd + 2, :]
                )
            nc.sync.dma_start(wp[:, 768:1792], wpack[:, 768:1792])
            nc.sync.dma_start(wp[:, 1792:WPACK_COLS], wpack[:, 1792:WPACK_COLS])
            wt = {nm: wp[:, off : off + P] for nm, off in W1_OFF.items()}
            wt.update({nm: wp[:, off : off + 2 * P] for nm, off in W2_OFF.items()})
            ident = wp[:, ID_OFF : ID_OFF + P]
            ones16 = wp[:, ONES_OFF : ONES_OFF + 16]

            qc_all, kc_all, nki_all, v_all = {}, {}, {}, {}

            def proj(b, pre=None):
                # qcat/kcat: [d, {r,i}, s] per-batch projection outputs
                qcat = qk.tile([P, 2, S], F32R, name="qcat", tag="qcat")
                kcat = qk.tile([P, 2, S], F32R, name="kcat", tag="kcat")
                nkiT = qk.tile([P, S], F32R, name="nkiT", tag="nkiT")
                vcat = vp.tile([P, 8, 258], F32R, name="vcat", tag="vcat")
                nc.sync.dma_start(
                    vcat[:, :, 256:258],
                    ones16.rearrange("p (a c) -> p a c", c=2),
                )
                for t2 in range(2):
                    cols = slice(t2 * 512, (t2 + 1) * 512)
                    if pre is not None and t2 in pre:
                        xt = pre[t2]
                    else:
                        xt = xp.tile([P, 6, 512], F32R, name="xt", tag="xt")
                        for dd in range(3):
                            nc.gpsimd.dma_start(
                                xt[:, 2 * dd : 2 * dd + 2, :],
                                xv[:, b * 2 + t2, 2 * dd : 2 * dd + 2, :],
                            )
                    xqr, xqi = xt[:, 0, :], xt[:, 1, :]
                    xkr, xki = xt[:, 2, :], xt[:, 3, :]
                    xvr, xvi = xt[:, 4, :], xt[:, 5, :]

                    # double-bank PSUM tiles: both complex halves land in one
                    # [P,1024] tile so the egress is a single fused DVE op
                    pq2 = ps.tile([P, 1024], F32, name="pq2", tag="m2")
                    nc.tensor.matmul(pq2[:, 0:512], wt["wqr"], xqr, start=True, stop=False)
                    nc.tensor.matmul(pq2[:, 0:512], wt["nwqi"], xqi, start=False, stop=True)
                    nc.tensor.matmul(pq2[:, 512:1024], wt["wqi"], xqr, start=True, stop=False)
                    nc.tensor.matmul(pq2[:, 512:1024], wt["wqr"], xqi, start=False, stop=True)
                    nc.vector.tensor_copy(
                        qcat[:, :, cols], pq2.rearrange("p (a c) -> p a c", a=2)
                    )

                    pk2 = ps.tile([P, 1024], F32, name="pk2", tag="m2")
                    nc.tensor.matmul(pk2[:, 0:512], wt["wkr"], xkr, start=True, stop=False)
                    nc.tensor.matmul(pk2[:, 0:512], wt["nwki"], xki, start=False, stop=True)
                    nc.tensor.matmul(pk2[:, 512:1024], wt["wki"], xkr, start=True, stop=False)
                    nc.tensor.matmul(pk2[:, 512:1024], wt["wkr"], xki, start=False, stop=True)
                    nc.vector.tensor_copy(
                        kcat[:, :, cols], pk2.rearrange("p (a c) -> p a c", a=2)
                    )

                    nc.gpsimd.tensor_scalar_mul(nkiT[:, cols], kcat[:, 1, cols], -1.0)

                    pv2 = ps.tile([P, 1024], F32, name="pv2", tag="m2")
                    for c2 in range(2):
                        for j in range(2):
                            cc = slice((c2 * 2 + j) * 128, (c2 * 2 + j + 1) * 128)
                            h = slice((c2 * 2 + j) * 256, (c2 * 2 + j + 1) * 256)
                            nc.tensor.matmul(pv2[:, h], xvr[:, cc], wt["vc1"], start=True, stop=False)
                            nc.tensor.matmul(pv2[:, h], xvi[:, cc], wt["vc2"], start=False, stop=True)
                    gc = t2 * 4
                    nc.vector.tensor_copy(
                        vcat[:, gc : gc + 4, 0:256],
                        pv2.rearrange("p (a c) -> p a c", a=4),
                    )
                qc_all[b] = qcat
                kc_all[b] = kcat
                nki_all[b] = nkiT
                v_all[b] = vcat

            # score tiles whose fused square runs on ACT (rest go to DVE);
            # chosen to balance ACT ~= DVE ~= 102us, both under PE ~115us
            ACT_SQ = {(0, 0), (0, 3), (0, 6), (1, 1), (1, 4), (1, 7)}

            def scores(b, qt, strip):
                qcat, kcat, nkiT = qc_all[b], kc_all[b], nki_all[b]
                qcols = slice(qt * 512, (qt + 1) * 512)
                for kc in range(8):
                    sl8 = qt * 8 + kc
                    kcols = slice(kc * 128, (kc + 1) * 128)
                    ps2 = ps.tile([P, 1024], F32, name="ps2", tag="m2")
                    nc.tensor.matmul(ps2[:, 0:512], kcat[:, 0, kcols], qcat[:, 0, qcols], start=True, stop=False)
                    nc.tensor.matmul(ps2[:, 0:512], nkiT[:, kcols], qcat[:, 1, qcols], start=False, stop=True)
                    nc.tensor.matmul(ps2[:, 512:1024], kcat[:, 1, kcols], qcat[:, 0, qcols], start=True, stop=False)
                    nc.tensor.matmul(ps2[:, 512:1024], kcat[:, 0, kcols], qcat[:, 1, qcols], start=False, stop=True)

                    # fused [1024] egress of both PSUM banks in one op: ACT
                    # squares in-flight; DVE plain-copies (a DVE op may read
                    # only one PSUM input) and Pool squares in SBUF. A Pool
                    # add then folds sr^2 + si^2 into the strip.
                    sq2 = sc.tile([P, 1024], F32, name="sq2", tag="sq2")
                    if (qt, kc) in ACT_SQ:
                        nc.scalar.square(sq2, ps2)
                        m2t = sq2
                    else:
                        nc.vector.tensor_copy(sq2, ps2)
                        m2t = sc.tile([P, 1024], F32, name="m2t", tag="m2t")
                        nc.gpsimd.tensor_mul(m2t, sq2, sq2)
                    nc.gpsimd.tensor_add(
                        strip[:, sl8, :], m2t[:, 0:512], m2t[:, 512:1024]
                    )

            def attn(b, qt, strip):
                vcat = v_all[b]
                ybuf = yp.tile([P, 4, 256], F32, name="ybuf", tag="ybuf")
                for qc in range(4):
                    qsub = slice(qc * 128, (qc + 1) * 128)
                    pso = po.tile([P, 258], F32, name="pso", tag="o")
                    for kc in range(8):
                        nc.tensor.matmul(
                            pso, strip[:, qt * 8 + kc, qsub], vcat[:, kc, :],
                            start=(kc == 0), stop=(kc == 7),
                        )
                    rec = rp.tile([P, 1], F32, name="rec", tag="rec")
                    nc.vector.reciprocal(rec, pso[:, 256:257])
                    # fold the 1/Z normalization into the PSUM egress copy
                    osb = op.tile([P, 256], F32R, name="osb", tag="osb")
                    nc.vector.tensor_scalar_mul(osb, pso[:, 0:256], rec)

                    ptp = pt.tile([P, 256], F32R, name="ptp", tag="tp")
                    nc.tensor.transpose(ptp[:, 0:128], osb[:, 0:128], ident)
                    nc.tensor.transpose(ptp[:, 128:256], osb[:, 128:256], ident)
                    ocat = op.tile([P, 2, 128], F32R, name="ocat", tag="ocat")
                    nc.vector.tensor_copy(ocat, ptp.rearrange("p (a c) -> p a c", a=2))

                    pyb = py.tile([P, 256], F32, name="pyb", tag="y")
                    nc.tensor.matmul(pyb, ocat[:, 0, :], wt["oc1"], start=True, stop=False)
                    nc.tensor.matmul(pyb, ocat[:, 1, :], wt["oc2"], start=False, stop=True)

                    nc.scalar.copy(ybuf[:, qc, :], pyb)
                base = b * S + qt * 512
                nc.sync.dma_start(
                    y[base : base + 512, :].rearrange("(a p) c -> p a c", p=P),
                    ybuf,
                )

            pend = []
            for b in range(B):
                proj(b, pre={0: xt00} if b == 0 else None)
                strip = sp.tile([P, 16, 512], F32R, name="strip", tag="strip")
                if b < B - 1:
                    scores(b, 0, strip)
                    if len(pend) > 2:
                        attn(*pend.pop(0))
                    scores(b, 1, strip)
                    if len(pend) > 1:
                        attn(*pend.pop(0))
                    nc.scalar.activation(strip, strip, AF.Sqrt, scale=1.0 / D)
                    nc.scalar.activation(strip, strip, AF.Exp)
                    pend += [(b, 0, strip), (b, 1, strip)]
                else:
                    # last batch: per-qt halves so attn can start sooner
                    for qt in range(2):
                        scores(b, qt, strip)
                        if pend:
                            attn(*pend.pop(0))
                        hs = strip[:, qt * 8 : qt * 8 + 8, :]
                        nc.scalar.activation(hs, hs, AF.Sqrt, scale=1.0 / D)
                        nc.scalar.activation(hs, hs, AF.Exp)
                        pend.append((b, qt, strip))
            for item in pend:
                attn(*item)
    nc.finalize()
    return nc


_NC = None


def _get_nc():
    global _NC
    if _NC is None:
        _NC = build_nc()
    return _NC


def make_in_maps(inputs):
    """Shard full inputs into 8 per-core input maps (head h -> core h)."""
    f = np.float32
    xT = {}
    for src_nm, nm in (("q_r", "xqr"), ("q_i", "xqi"), ("k_r", "xkr"),
                       ("k_i", "xki"), ("v_r", "xvr"), ("v_i", "xvi")):
        xT[nm] = np.asarray(inputs[src_nm], f).reshape(BS, D).T
    # xall layout: [P, t(8), nm(6), 512]
    stack = np.stack([xT[nm].reshape(P, 8, 512) for nm in X_NAMES], axis=2)
    xall = np.ascontiguousarray(stack.reshape(P, 8 * 6 * 512))

    Wq_r = np.asarray(inputs["Wq_r"], f)
    Wq_i = np.asarray(inputs["Wq_i"], f)
    Wk_r = np.asarray(inputs["Wk_r"], f)
    Wk_i = np.asarray(inputs["Wk_i"], f)
    Wv_r = np.asarray(inputs["Wv_r"], f)
    Wv_i = np.asarray(inputs["Wv_i"], f)
    Wo_r = np.asarray(inputs["Wo_r"], f)
    Wo_i = np.asarray(inputs["Wo_i"], f)

    in_maps = []
    for h in range(H):
        sl = slice(h * D, (h + 1) * D)
        w = {
            "wqr": Wq_r[sl].T, "wqi": Wq_i[sl].T, "nwqi": -Wq_i[sl].T,
            "wkr": Wk_r[sl].T, "wki": Wk_i[sl].T, "nwki": -Wk_i[sl].T,
            "vc1": np.concatenate([Wv_r[sl].T, Wv_i[sl].T], axis=1),
            "vc2": np.concatenate([-Wv_i[sl].T, Wv_r[sl].T], axis=1),
            "oc1": np.concatenate([Wo_r[:, sl].T, Wo_i[:, sl].T], axis=1),
            "oc2": np.concatenate([-Wo_i[:, sl].T, Wo_r[:, sl].T], axis=1),
        }
        wpack = np.zeros((P, WPACK_COLS), f)
        for nm, off in W1_OFF.items():
            wpack[:, off : off + P] = w[nm]
        for nm, off in W2_OFF.items():
            wpack[:, off : off + 2 * P] = w[nm]
        wpack[:, ID_OFF : ID_OFF + P] = np.eye(P, dtype=f)
        wpack[:, ONES_OFF : ONES_OFF + 16] = 1.0
        in_maps.append({"xall": xall, "wpack": wpack})
    return in_maps


def run(inputs, trace=False):
    nc = _get_nc()
    in_maps = make_in_maps(inputs)
    res = run_bass_kernel_spmd(nc, in_maps, core_ids=list(range(H)), trace=trace)
    ysum = np.zeros((BS, 2 * P), np.float64)
    for r in res.results:
        ysum += r["y"].astype(np.float64)
    yr = ysum[:, :P].reshape(B, S, D)
    yi = ysum[:, P:].reshape(B, S, D)
    out = (yr + 1j * yi).astype(np.complex64)
    return out, res


def kernel(**inputs):
    out, _ = run(inputs, trace=False)
    return out



# revision 14
# speedup vs baseline: 1.1501x; 1.1501x over previous
"""Complex-valued multi-head attention (B=4, S=1024, D=128, H=8) on 8 TRN2 cores.

Sharding: tensor-parallel over heads -- one head per NeuronCore. Each core
computes its head's complex Q/K/V projections, complex-magnitude-softmax
attention, and the partial W_o projection for its head slice; the host sums
the 8 partial outputs (the W_o contraction over heads).

Per-core dataflow (fp32 storage, float32r matmuls -- 1 cycle/row on the PE
at moving free-dim >= 256, vs 4 for fp32):
  - Host packs x^T tensors [D, B*S] and per-head weight slices (transposed,
    pre-negated / concatenated) into two DRAM tensors (xall, wpack).
  - Q^T, K^T computed as [d, {r,i}, s] via lhsT=W^T, rhs=x^T; -K_i^T by a
    Pool negate.  V computed natural [s, d] as [V_r | V_i] rows.
  - Scores computed transposed S^T[k, q] so E = exp(|s|/sqrt(D)) lands in
    the layout attn@V needs.  |s|^2 = sr^2 + si^2 egress is split per-tile:
    most tiles ACT-square sr straight into the strip while DVE copies si
    out (Pool squares+adds); the rest keep ACT free (DVE copies both banks,
    Pool squares both and adds) so sqrt/exp passes don't stall the PE.
  - attn@V computed TRANSPOSED: O^T[d, q] accumulates via lhsT=V-chunk
    (stationary), rhs=E-strip; the softmax normalizer Z[q] comes from tiny
    [q,1] matmuls with the E-chunk stationary against a ones column (PE
    cost ~4 cycles each).  This kills the PE transposes and the ocat/osb
    staging copies of the old layout.
  - W_o applied with lhsT=O^T slices; 1/Z folded into the final PSUM->SBUF
    egress via per-partition tensor_scalar.
  - Software pipelining: each (b, qt) unit's attention is deferred two units
    and interleaved between the next batch's score halves, so PE always has
    score/projection matmuls to run while ACT/DVE work through a unit's
    softmax chain.
"""

import numpy as np

import concourse.bacc as bacc
import concourse.mybir as mybir
import concourse.tile as tile
from concourse.bass_utils import run_bass_kernel_spmd

B, S, D, H = 4, 1024, 128, 8
BS = B * S
P = 128
F32 = mybir.dt.float32
F32R = mybir.dt.float32r

X_NAMES = ("xqr", "xqi", "xkr", "xki", "xvr", "xvi")
W1_NAMES = ("wqr", "wqi", "nwqi", "wkr", "wki", "nwki")
W2_NAMES = ("vc1", "vc2", "oc1", "oc2")


W1_OFF = {nm: i * P for i, nm in enumerate(W1_NAMES)}
W2_OFF = {nm: 6 * P + i * 2 * P for i, nm in enumerate(W2_NAMES)}
ID_OFF = 6 * P + 4 * 2 * P          # 1792
ONES_OFF = ID_OFF + P               # 1920
WPACK_COLS = ONES_OFF + 16          # 1936


def build_nc():
    nc = bacc.Bacc()
    xall = nc.dram_tensor("xall", [P, 8 * 6 * 512], F32R, kind="ExternalInput")
    wpack = nc.dram_tensor("wpack", [P, WPACK_COLS], F32R, kind="ExternalInput")
    y = nc.dram_tensor("y", [BS, 2 * P], F32, kind="ExternalOutput")
    xv = xall.rearrange("p (t n c) -> p t n c", t=8, n=6)

    AF = mybir.ActivationFunctionType
    MUL = mybir.AluOpType.mult

    with tile.TileContext(nc) as tc:
        with (
            tc.tile_pool(name="consts", bufs=1) as consts,
            tc.tile_pool(name="xp", bufs=3) as xp,
            tc.tile_pool(name="qk", bufs=2) as qk,
            tc.tile_pool(name="vp", bufs=2) as vp,
            tc.tile_pool(name="sp", bufs=2) as sp,
            tc.tile_pool(name="sc", bufs=4) as sc,
            tc.tile_pool(name="ot", bufs=2) as otp,
            tc.tile_pool(name="yp", bufs=2) as yp,
            tc.tile_pool(name="rp", bufs=4) as rp,
            tc.tile_pool(name="ps", bufs=2, space="PSUM") as ps,
            tc.tile_pool(name="po", bufs=1, space="PSUM") as po,
            tc.tile_pool(name="pz", bufs=1, space="PSUM") as pzp,
            tc.tile_pool(name="py", bufs=1, space="PSUM") as pyp,
        ):
            wp = consts.tile([P, WPACK_COLS], F32R, name="wp")
            nc.sync.dma_start(wp[:, 0:768], wpack[:, 0:768])
            xt00 = xp.tile([P, 6, 512], F32R, name="xt", tag="xt")
            for dd in range(3):
                nc.sync.dma_start(
                    xt00[:, 2 * dd : 2 * dd + 2, :], xv[:, 0, 2 * dd : 2 * dd + 2, :]
                )
            nc.sync.dma_start(wp[:, 768:1792], wpack[:, 768:1792])
            nc.sync.dma_start(wp[:, 1792:WPACK_COLS], wpack[:, 1792:WPACK_COLS])
            wt = {nm: wp[:, off : off + P] for nm, off in W1_OFF.items()}
            wt.update({nm: wp[:, off : off + 2 * P] for nm, off in W2_OFF.items()})
            ones2 = wp[:, ONES_OFF : ONES_OFF + 2]
            ones16 = wp[:, ONES_OFF : ONES_OFF + 16]

            qc_all, kc_all, nki_all, v_all = {}, {}, {}, {}

            def proj(b, pre=None):
                # qcat/kcat: [d, {r,i}, s] per-batch projection outputs
                qcat = qk.tile([P, 2, S], F32R, name="qcat", tag="qcat")
                kcat = qk.tile([P, 2, S], F32R, name="kcat", tag="kcat")
                nkiT = qk.tile([P, S], F32R, name="nkiT", tag="nkiT")
                vcat = vp.tile([P, 8, 256], F32R, name="vcat", tag="vcat")
                for t2 in range(2):
                    cols = slice(t2 * 512, (t2 + 1) * 512)
                    if pre is not None and t2 in pre:
                        xt = pre[t2]
                    else:
                        xt = xp.tile([P, 6, 512], F32R, name="xt", tag="xt")
                        for dd in range(3):
                            nc.sync.dma_start(
                                xt[:, 2 * dd : 2 * dd + 2, :],
                                xv[:, b * 2 + t2, 2 * dd : 2 * dd + 2, :],
                            )
                    xqr, xqi = xt[:, 0, :], xt[:, 1, :]
                    xkr, xki = xt[:, 2, :], xt[:, 3, :]
                    xvr, xvi = xt[:, 4, :], xt[:, 5, :]

                    pqr = ps.tile([P, 512], F32, name="pqr", tag="ma")
                    nc.tensor.matmul(pqr, wt["wqr"], xqr, start=True, stop=False)
                    nc.tensor.matmul(pqr, wt["nwqi"], xqi, start=False, stop=True)
                    nc.vector.tensor_copy(qcat[:, 0, cols], pqr)
                    pqi = ps.tile([P, 512], F32, name="pqi", tag="mb")
                    nc.tensor.matmul(pqi, wt["wqi"], xqr, start=True, stop=False)
                    nc.tensor.matmul(pqi, wt["wqr"], xqi, start=False, stop=True)
                    nc.vector.tensor_copy(qcat[:, 1, cols], pqi)

                    pkr = ps.tile([P, 512], F32, name="pkr", tag="ma")
                    nc.tensor.matmul(pkr, wt["wkr"], xkr, start=True, stop=False)
                    nc.tensor.matmul(pkr, wt["nwki"], xki, start=False, stop=True)
                    nc.vector.tensor_copy(kcat[:, 0, cols], pkr)
                    pki = ps.tile([P, 512], F32, name="pki", tag="mb")
                    nc.tensor.matmul(pki, wt["wki"], xkr, start=True, stop=False)
                    nc.tensor.matmul(pki, wt["wkr"], xki, start=False, stop=True)
                    nc.vector.tensor_copy(kcat[:, 1, cols], pki)

                    nc.gpsimd.tensor_scalar_mul(nkiT[:, cols], kcat[:, 1, cols], -1.0)

                    for c2 in range(2):
                        gc = t2 * 4 + c2 * 2
                        pv = ps.tile([P, 512], F32, name="pv", tag="ma" if c2 == 0 else "mb")
                        for j in range(2):
                            cc = slice((c2 * 2 + j) * 128, (c2 * 2 + j + 1) * 128)
                            h = slice(j * 256, (j + 1) * 256)
                            nc.tensor.matmul(pv[:, h], xvr[:, cc], wt["vc1"], start=True, stop=False)
                            nc.tensor.matmul(pv[:, h], xvi[:, cc], wt["vc2"], start=False, stop=True)
                        nc.vector.tensor_copy(
                            vcat[:, gc : gc + 2, :],
                            pv.rearrange("p (a c) -> p a c", a=2),
                        )
                qc_all[b] = qcat
                kc_all[b] = kcat
                nki_all[b] = nkiT
                v_all[b] = vcat

            # score tiles where ACT squares sr in-flight (DVE handles si);
            # the rest are ACT-free (DVE copies, Pool squares) so attention
            # sqrt/exp passes don't stall the score pipeline
            DVE_SQ = {(1, 1), (1, 3), (1, 5)}

            def scores(b, qt, strip):
                qcat, kcat, nkiT = qc_all[b], kc_all[b], nki_all[b]
                qcols = slice(qt * 512, (qt + 1) * 512)
                for kc in range(8):
                    sl8 = qt * 8 + kc
                    kcols = slice(kc * 128, (kc + 1) * 128)
                    psr = ps.tile([P, 512], F32, name="psr", tag="ma")
                    nc.tensor.matmul(psr, kcat[:, 0, kcols], qcat[:, 0, qcols], start=True, stop=False)
                    nc.tensor.matmul(psr, nkiT[:, kcols], qcat[:, 1, qcols], start=False, stop=True)
                    psi = ps.tile([P, 512], F32, name="psi", tag="mb")
                    nc.tensor.matmul(psi, kcat[:, 1, kcols], qcat[:, 0, qcols], start=True, stop=False)
                    nc.tensor.matmul(psi, kcat[:, 0, kcols], qcat[:, 1, qcols], start=False, stop=True)

                    cpi = sc.tile([P, 512], F32, name="cpi", tag="cpi")
                    nc.vector.tensor_copy(cpi, psi)
                    t2s = sc.tile([P, 512], F32, name="t2s", tag="t2s")
                    nc.gpsimd.tensor_mul(t2s, cpi, cpi)
                    if (qt, kc) in DVE_SQ:
                        cpr = sc.tile([P, 512], F32, name="cpr", tag="cpr")
                        nc.vector.tensor_copy(cpr, psr)
                        nc.gpsimd.tensor_mul(strip[:, sl8, :], cpr, cpr)
                    else:
                        nc.scalar.square(strip[:, sl8, :], psr)
                    nc.gpsimd.tensor_add(strip[:, sl8, :], strip[:, sl8, :], t2s)

            def attn(b, qt, strip):
                vcat = v_all[b]
                # Z[q] per qc-chunk: tiny [q,1] matmuls, E-chunk stationary.
                # One start=True pending-zeroes the whole bank; later columns
                # rely on it (start=False + skip_group_check).
                pzt = pzp.tile([P, 8], F32, name="pzt", tag="z")
                for qc in range(4):
                    qsub = slice(qc * 128, (qc + 1) * 128)
                    for kc in range(8):
                        nc.tensor.matmul(
                            pzt[:, 2 * qc : 2 * qc + 2],
                            strip[:, qt * 8 + kc, qsub],
                            ones2,
                            start=(qc == 0 and kc == 0),
                            stop=(kc == 7),
                            skip_group_check=True,
                        )
                rec4 = rp.tile([P, 8], F32, name="rec4", tag="rec")
                nc.vector.reciprocal(rec4, pzt)

                # O^T[d, q] accumulation, V-chunk stationary; both complex
                # halves in one double-bank PSUM tile for a fused egress
                po2 = po.tile([P, 1024], F32, name="po2", tag="o")
                for kc in range(8):
                    nc.tensor.matmul(
                        po2[:, 0:512], vcat[:, kc, 0:128], strip[:, qt * 8 + kc, :],
                        start=(kc == 0), stop=(kc == 7),
                    )
                for kc in range(8):
                    nc.tensor.matmul(
                        po2[:, 512:1024], vcat[:, kc, 128:256], strip[:, qt * 8 + kc, :],
                        start=(kc == 0), stop=(kc == 7),
                    )
                oT = otp.tile([P, 2, 512], F32R, name="oT", tag="oT")
                nc.vector.tensor_copy(oT, po2.rearrange("p (a c) -> p a c", a=2))

                ybuf = yp.tile([P, 4, 256], F32, name="ybuf", tag="ybuf")
                pyt = pyp.tile([P, 512], F32, name="pyt", tag="y")
                for qc in range(4):
                    qsub = slice(qc * 128, (qc + 1) * 128)
                    half = slice((qc % 2) * 256, (qc % 2) * 256 + 256)
                    nc.tensor.matmul(pyt[:, half], oT[:, 0, qsub], wt["oc1"], start=True, stop=False)
                    nc.tensor.matmul(pyt[:, half], oT[:, 1, qsub], wt["oc2"], start=False, stop=True)
                    nc.vector.tensor_scalar_mul(
                        ybuf[:, qc, :], pyt[:, half], rec4[:, 2 * qc : 2 * qc + 1]
                    )
                base = b * S + qt * 512
                nc.sync.dma_start(
                    y[base : base + 512, :].rearrange("(a p) c -> p a c", p=P),
                    ybuf,
                )

            pend = []
            for b in range(B):
                proj(b, pre={0: xt00} if b == 0 else None)
                strip = sp.tile([P, 16, 512], F32R, name="strip", tag="strip")
                if b < B - 1:
                    scores(b, 0, strip)
                    if len(pend) > 2:
                        attn(*pend.pop(0))
                    scores(b, 1, strip)
                    if len(pend) > 1:
                        attn(*pend.pop(0))
                    nc.scalar.activation(strip, strip, AF.Sqrt, scale=1.0 / D)
                    nc.scalar.activation(strip, strip, AF.Exp)
                    pend += [(b, 0, strip), (b, 1, strip)]
                else:
                    # last batch: per-qt halves so attn can start sooner
                    for qt in range(2):
                        scores(b, qt, strip)
                        if pend:
                            attn(*pend.pop(0))
                        hs = strip[:, qt * 8 : qt * 8 + 8, :]
                        nc.scalar.activation(hs, hs, AF.Sqrt, scale=1.0 / D)
                        nc.scalar.activation(hs, hs, AF.Exp)
                        pend.append((b, qt, strip))
            for item in pend:
                attn(*item)
    nc.finalize()
    return nc


_NC = None


def _get_nc():
    global _NC
    if _NC is None:
        _NC = build_nc()
    return _NC


def make_in_maps(inputs):
    """Shard full inputs into 8 per-core input maps (head h -> core h)."""
    f = np.float32
    xT = {}
    for src_nm, nm in (("q_r", "xqr"), ("q_i", "xqi"), ("k_r", "xkr"),
                       ("k_i", "xki"), ("v_r", "xvr"), ("v_i", "xvi")):
        xT[nm] = np.asarray(inputs[src_nm], f).reshape(BS, D).T
    # xall layout: [P, t(8), nm(6), 512]
    stack = np.stack([xT[nm].reshape(P, 8, 512) for nm in X_NAMES], axis=2)
    xall = np.ascontiguousarray(stack.reshape(P, 8 * 6 * 512))

    Wq_r = np.asarray(inputs["Wq_r"], f)
    Wq_i = np.asarray(inputs["Wq_i"], f)
    Wk_r = np.asarray(inputs["Wk_r"], f)
    Wk_i = np.asarray(inputs["Wk_i"], f)
    Wv_r = np.asarray(inputs["Wv_r"], f)
    Wv_i = np.asarray(inputs["Wv_i"], f)
    Wo_r = np.asarray(inputs["Wo_r"], f)
    Wo_i = np.asarray(inputs["Wo_i"], f)

    in_maps = []
    for h in range(H):
        sl = slice(h * D, (h + 1) * D)
        w = {
            "wqr": Wq_r[sl].T, "wqi": Wq_i[sl].T, "nwqi": -Wq_i[sl].T,
            "wkr": Wk_r[sl].T, "wki": Wk_i[sl].T, "nwki": -Wk_i[sl].T,
            "vc1": np.concatenate([Wv_r[sl].T, Wv_i[sl].T], axis=1),
            "vc2": np.concatenate([-Wv_i[sl].T, Wv_r[sl].T], axis=1),
            "oc1": np.concatenate([Wo_r[:, sl].T, Wo_i[:, sl].T], axis=1),
            "oc2": np.concatenate([-Wo_i[:, sl].T, Wo_r[:, sl].T], axis=1),
        }
        wpack = np.zeros((P, WPACK_COLS), f)
        for nm, off in W1_OFF.items():
            wpack[:, off : off + P] = w[nm]
        for nm, off in W2_OFF.items():
            wpack[:, off : off + 2 * P] = w[nm]
        wpack[:, ID_OFF : ID_OFF + P] = np.eye(P, dtype=f)
        wpack[:, ONES_OFF : ONES_OFF + 16] = 1.0
        in_maps.append({"xall": xall, "wpack": wpack})
    return in_maps


def run(inputs, trace=False):
    nc = _get_nc()
    in_maps = make_in_maps(inputs)
    res = run_bass_kernel_spmd(nc, in_maps, core_ids=list(range(H)), trace=trace)
    ysum = np.zeros((BS, 2 * P), np.float64)
    for r in res.results:
        ysum += r["y"].astype(np.float64)
    yr = ysum[:, :P].reshape(B, S, D)
    yi = ysum[:, P:].reshape(B, S, D)
    out = (yr + 1j * yi).astype(np.complex64)
    return out, res


def kernel(**inputs):
    out, _ = run(inputs, trace=False)
    return out
